# revision 1
# baseline (speedup 1.0000x reference)
"""Distributed causal multi-head attention for Trainium2 (8 NeuronCores).

Problem (hardcoded): x[2, 2048, 1024], 16 heads, head_dim 64, causal
softmax(QK^T/8)V then out-proj with bias. f32 in/out.

Sharding: data parallel on batch (cores 0-3 -> batch 0, 4-7 -> batch 1),
tensor parallel on heads within each group of 4 (4 heads per core).
Each core:
  - computes Q^T,K^T (head pairs packed to 128 partitions), V for its 4 heads
  - scores transposed S^T[k,q] = K Q^T so the softmax denominator comes out
    of the PE via an appended ones-column on V (no partition reductions)
  - exp without max-subtraction (scores are O(2), safe in fp32/bf16)
  - causal mask applied post-exp as a 0/1 bf16 multiply (DVE 4x mode)
  - ctx^T accumulated per q-chunk, normalized with 1/den partition-broadcast
  - AllGather of ctx^T bf16 [256,2048] within the 4-core group
  - column-parallel out-proj: outT[oc,q] = Wo[:,oc]^T ctxT + bo[oc]
Host assembles out[b, :, oc_slice] from each core's outT.

All matmuls bf16 (fp32 PSUM accumulation): measured end-to-end rel err
(Frobenius) ~3e-3 vs the f32 reference.
"""

import numpy as np
import ml_dtypes

from concourse import bass, bacc, mybir
from concourse import tile
from concourse.bass_utils import run_bass_kernel_spmd

BF16 = mybir.dt.bfloat16
F32 = mybir.dt.float32
Act = mybir.ActivationFunctionType

B, S, D = 2, 2048, 1024
H, HD = 16, 64
NCORES = 8
GROUP = 4            # cores per batch group
HPC = H // GROUP     # 4 heads per core
CW = HPC * HD        # 256 columns per core
QC = 512             # q-chunk width
KC = 128             # k-chunk width
NQ = S // QC         # 4
NKC = S // KC        # 16
KPQ = QC // KC       # 4 k-chunks per q-chunk
DCH = D // 128       # 8 contraction chunks of 128

_CACHE = {}


def _build_bass(reps=1):
    nc = bacc.Bacc(
        "TRN2", target_bir_lowering=False, debug=False, num_devices=NCORES
    )
    # Tile under-syncs readers of async collective outputs (readback DMAs can
    # fire before the gather lands); completion waits are attached post-Tile
    _ccs = []
    _rds = []

    # per-core external inputs (same shapes on every core: SPMD)
    xT = nc.declare_dram_parameter("xT", [D, S], BF16, isOutput=False)
    wq = nc.declare_dram_parameter("wq", [D, CW], BF16, isOutput=False)
    wk = nc.declare_dram_parameter("wk", [D, CW], BF16, isOutput=False)
    wv = nc.declare_dram_parameter("wv", [D, CW], BF16, isOutput=False)
    wo = nc.declare_dram_parameter("wo", [D, CW], BF16, isOutput=False)
    bo = nc.declare_dram_parameter("bo", [CW, 1], F32, isOutput=False)
    msk = nc.declare_dram_parameter("msk", [128, KPQ, QC], BF16, isOutput=False)
    vones = nc.declare_dram_parameter("vones", [128, NKC, HPC, 1], BF16, isOutput=False)
    # selector for den broadcast: bc[m,q] = sum_k sel33[k,m]*den_pair[k,q]
    sel33 = nc.declare_dram_parameter("sel33", [33, 128], BF16, isOutput=False)
    outT = nc.declare_dram_parameter("outT", [CW, S], F32, isOutput=True)

    with tile.TileContext(nc) as tc:
        with tc.tile_pool(name="dram", bufs=1, space="DRAM") as dram:
            # one gather per head-pair so comm overlaps the next pair's
            # attention. Shared addr_space needs >4-core groups; Local here.
            cc_in = [dram.tile([128, S], BF16, name=f"cc_in{p}") for p in range(2)]
            cc_out = [dram.tile([GROUP * 128, S], BF16, name=f"cc_out{p}")
                      for p in range(2)]

            with tc.tile_pool(name="persist", bufs=1) as pp:
                # lives across the whole kernel: ~92 KB/partition
                wq_sb = pp.tile([128, DCH, CW], BF16, tag="wq_sb")
                wk_sb = pp.tile([128, DCH, CW], BF16, tag="wk_sb")
                wv_sb = pp.tile([128, DCH, CW], BF16, tag="wv_sb")
                wo_sb = pp.tile([128, DCH, CW], BF16, tag="wo_sb")
                bo_sb = pp.tile([128, CW // 128, 1], F32, tag="bo_sb")
                msk_sb = pp.tile([128, KPQ, QC], BF16, tag="msk_sb")
                qT_sb = pp.tile([128, 2, S], BF16, tag="qT_sb")
                kT_sb = pp.tile([128, 2, S], BF16, tag="kT_sb")
                v_aug = pp.tile([128, NKC, HPC, HD + 1], BF16, tag="v_aug")
                ctxu0 = pp.tile([128, S], F32, tag="ctxu0")
                ctxu1 = pp.tile([128, S], F32, tag="ctxu1")
                # den per pair: head 2p at partition 0, head 2p+1 at partition
                # 32 (ACT writes must start at multiples of 32); rows 1-31 are
                # zeroed so the K=33 selector matmul can broadcast both heads
                # to output partitions 0-63 / 64-127 in one instruction
                den_pair = [pp.tile([33, S], BF16, tag=f"den{p}", name=f"den{p}")
                            for p in range(2)]
                sel_sb = pp.tile([33, 128], BF16, tag="sel_sb")
                ctxu_pair = [ctxu0, ctxu1]
                for p in range(2):
                    nc.vector.memset(den_pair[p][:], 0.0)

                # DMA order matters for startup latency: x first so the
                # projection matmuls can start streaming, wo/bo last
                xT_sb = pp.tile([128, DCH, S], BF16, tag="xT_sb")
                for c in range(DCH):
                    nc.sync.dma_start(xT_sb[:, c, :], xT[c * 128:(c + 1) * 128, :])
                for w_sb, w in ((wq_sb, wq), (wk_sb, wk), (wv_sb, wv)):
                    for c in range(DCH):
                        nc.sync.dma_start(w_sb[:, c, :], w[c * 128:(c + 1) * 128, :])
                nc.sync.dma_start(msk_sb[:], msk[:])
                # ones column of V_aug comes from the host: keeps the V
                # PSUM->SBUF copy to a single (PE) sync wait
                nc.sync.dma_start(v_aug[:, :, :, HD:HD + 1], vones[:])
                nc.sync.dma_start(sel_sb[:], sel33[:])
                for c in range(DCH):
                    nc.sync.dma_start(wo_sb[:, c, :], wo[c * 128:(c + 1) * 128, :])
                for o in range(CW // 128):
                    nc.sync.dma_start(bo_sb[:, o, :], bo[o * 128:(o + 1) * 128, :])

              # reps>1 repeats the whole computation for differential
              # wall-clock timing (no NTFF profiling path in this setup)
                def _emit_once():
                    # All PSUM pools coexist (phases interleave): 2+4+2 banks
                    with tc.tile_pool(name="proj_ps", bufs=2, space="PSUM") as projp, \
                         tc.tile_pool(name="sc_ps", bufs=2, space="PSUM") as scp, \
                         tc.tile_pool(name="ctbc_ps", bufs=2, space="PSUM") as ctp, \
                         tc.tile_pool(name="es_pool", bufs=NKC // 2 + 2) as esp, \
                         tc.tile_pool(name="norm", bufs=2) as np_pool:

                        def proj_qk(pair):
                            for w_sb, dst in ((wq_sb, qT_sb), (wk_sb, kT_sb)):
                                for j in range(NQ):
                                    ps = projp.tile([128, QC], F32, tag="proj")
                                    for c in range(DCH):
                                        nc.tensor.matmul(
                                            ps[:],
                                            w_sb[:, c, pair * 128:(pair + 1) * 128],
                                            xT_sb[:, c, j * QC:(j + 1) * QC],
                                            start=(c == 0),
                                            stop=(c == DCH - 1),
                                        )
                                    nc.vector.tensor_copy(
                                        dst[:, pair, j * QC:(j + 1) * QC], ps[:]
                                    )

                        def proj_v(pair):
                            # V for this pair's 2 heads: [tok, 2*64]
                            for t in range(NKC):
                                ps = projp.tile([128, QC], F32, tag="proj")
                                for c in range(DCH):
                                    nc.tensor.matmul(
                                        ps[:, 0:128],
                                        xT_sb[:, c, t * 128:(t + 1) * 128],
                                        wv_sb[:, c, pair * 128:(pair + 1) * 128],
                                        start=(c == 0),
                                        stop=(c == DCH - 1),
                                    )
                                nc.vector.tensor_copy(
                                    v_aug[:, t, 2 * pair:2 * pair + 2, 0:HD],
                                    ps[:, 0:128].rearrange("p (h w) -> p h w", h=2),
                                )

                        def attn_head(h):
                            pair, hh = h // 2, h % 2
                            row = hh * 64
                            for j in range(NQ):
                                nkc = (j + 1) * KPQ
                                qs = slice(j * QC, (j + 1) * QC)
                                es_tiles = []
                                for c0 in range(0, nkc, 2):
                                    # two k-chunks share one 2-bank PSUM tile
                                    # -> one exp instruction
                                    st = scp.tile([128, 2, QC], F32, tag="st")
                                    for i in range(2):
                                        c = c0 + i
                                        nc.tensor.matmul(
                                            st[:, i, :],
                                            kT_sb[row:row + 64, pair, c * KC:(c + 1) * KC],
                                            qT_sb[row:row + 64, pair, qs],
                                            start=True, stop=True,
                                        )
                                    es = esp.tile([128, 2, QC], BF16, tag="es")
                                    nc.scalar.activation(es[:], st[:], Act.Exp, scale=0.125)
                                    if c0 >= j * KPQ:
                                        r = c0 - j * KPQ
                                        nc.vector.tensor_mul(
                                            es[:], es[:], msk_sb[:, r:r + 2, :]
                                        )
                                    es_tiles.append(es)
                                ct = ctp.tile([HD + 1, QC], F32, tag="ct")
                                for c in range(nkc):
                                    nc.tensor.matmul(
                                        ct[:],
                                        v_aug[:, c, h, :],
                                        es_tiles[c // 2][:, c % 2, :],
                                        start=(c == 0),
                                        stop=(c == nkc - 1),
                                    )
                                nc.vector.tensor_copy(
                                    ctxu_pair[pair][row:row + 64, qs], ct[0:HD, :]
                                )
                                nc.vector.tensor_copy(
                                    den_pair[pair][hh * 32:hh * 32 + 1, qs],
                                    ct[HD:HD + 1, :],
                                )

                        def norm_cc(pair):
                            ctxn = np_pool.tile([128, S], BF16, tag="ctxn")
                            for j in range(NQ):
                                qs = slice(j * QC, (j + 1) * QC)
                                bc = ctp.tile([128, QC], F32, tag="ct")
                                nc.tensor.matmul(
                                    bc[:], sel_sb[:], den_pair[pair][:, qs],
                                    start=True, stop=True,
                                )
                                rb = np_pool.tile([128, QC], F32, tag="rb")
                                nc.vector.reciprocal(rb[:], bc[:])
                                nc.vector.tensor_mul(
                                    ctxn[:, qs], ctxu_pair[pair][:, qs], rb[:]
                                )
                            nc.sync.dma_start(cc_in[pair][:], ctxn[:])
                            _ccs.append(nc.gpsimd.collective_compute(
                                "AllGather",
                                mybir.AluOpType.bypass,
                                replica_groups=[[0, 1, 2, 3], [4, 5, 6, 7]],
                                ins=[cc_in[pair].opt()],
                                outs=[cc_out[pair].opt()],
                            ))

                        # conservative sequential ordering: overlapped
                        # variants showed intermittent collective races
                        proj_qk(0)
                        proj_qk(1)
                        proj_v(0)
                        proj_v(1)
                        attn_head(0)
                        attn_head(1)
                        norm_cc(0)
                        attn_head(2)
                        attn_head(3)
                        norm_cc(1)

                    # ---- out-proj: outT[oc, q] = Wo[:, oc]^T ctxT + bo ----
                    # gather #p holds global ctx chunks {2r+p}; accumulate the
                    # pair-0 chunks first so they overlap gather #1
                    with tc.tile_pool(name="cpool", bufs=1) as cp, \
                         tc.tile_pool(name="out_ps", bufs=4, space="PSUM") as outp, \
                         tc.tile_pool(name="out_sb", bufs=3) as outs:
                        ctxT_sb = cp.tile([128, DCH, S], BF16, tag="ctxT_sb")
                        for p in range(2):
                            for r in range(GROUP):
                                _rds.append((nc.sync.dma_start(
                                    ctxT_sb[:, 2 * r + p, :],
                                    cc_out[p][r * 128:(r + 1) * 128, :],
                                ), p))
                        chunk_order = [2 * r for r in range(GROUP)] + \
                                      [2 * r + 1 for r in range(GROUP)]
                        for o in range(CW // 128):
                            for j in range(NQ):
                                ps = outp.tile([128, QC], F32, tag="ops")
                                for ci, c in enumerate(chunk_order):
                                    nc.tensor.matmul(
                                        ps[:],
                                        wo_sb[:, c, o * 128:(o + 1) * 128],
                                        ctxT_sb[:, c, j * QC:(j + 1) * QC],
                                        start=(ci == 0),
                                        stop=(ci == DCH - 1),
                                    )
                                ot = outs.tile([128, QC], F32, tag="ot")
                                nc.scalar.activation(
                                    ot[:], ps[:], Act.Identity, bias=bo_sb[:, o, :]
                                )
                                nc.sync.dma_start(
                                    outT[o * 128:(o + 1) * 128, j * QC:(j + 1) * QC],
                                    ot[:],
                                )

                for _rep in range(reps):
                    _emit_once()
    upd = _ccs[0].ins.sync_info.on_update[0]
    cc_done_sem = bass.SemaphoreHandle(upd.ant_name, upd.id)
    per_rep = len(_rds) // reps
    for i, (rd, p) in enumerate(_rds):
        rep = i // per_rep
        # check=False: wait slot may be taken; bacc splits into event sems
        rd.wait_op(cc_done_sem, 2 * rep + p + 1, "sem-ge", check=False)
    nc.compile()
    return nc


def _causal_mask():
    # msk[kp, r, qf] = 1 where (r*128 + kp) <= qf else 0  (keep k <= q)
    kp = np.arange(128)[:, None, None]
    r = np.arange(KPQ)[None, :, None]
    qf = np.arange(QC)[None, None, :]
    return (r * 128 + kp <= qf).astype(ml_dtypes.bfloat16)


def _in_maps(x, Wq, Wk, Wv, Wo, bo):
    bf = ml_dtypes.bfloat16
    msk = _causal_mask()
    sel33 = np.zeros((33, 128), dtype=bf)
    sel33[0, 0:64] = 1.0
    sel33[32, 64:128] = 1.0
    xT = [np.ascontiguousarray(x[b].T).astype(bf) for b in range(B)]
    maps = []
    for c in range(NCORES):
        b, g = c // GROUP, c % GROUP
        cs = slice(g * CW, (g + 1) * CW)
        maps.append({
            "xT": xT[b],
            "wq": np.ascontiguousarray(Wq[:, cs]).astype(bf),
            "wk": np.ascontiguousarray(Wk[:, cs]).astype(bf),
            "wv": np.ascontiguousarray(Wv[:, cs]).astype(bf),
            "wo": np.ascontiguousarray(Wo[:, cs]).astype(bf),
            "bo": np.ascontiguousarray(bo[cs, None]).astype(np.float32),
            "msk": msk,
            "vones": np.ones((128, NKC, HPC, 1), dtype=bf),
            "sel33": sel33,
        })
    return maps


def kernel(x, Wq, Wk, Wv, Wo, bo, _trace=False):
    x = np.asarray(x, dtype=np.float32)
    Wq, Wk, Wv, Wo, bo = (np.asarray(a, dtype=np.float32) for a in (Wq, Wk, Wv, Wo, bo))
    if "nc" not in _CACHE:
        _CACHE["nc"] = _build_bass()
    nc = _CACHE["nc"]
    res = run_bass_kernel_spmd(
        nc, _in_maps(x, Wq, Wk, Wv, Wo, bo), list(range(NCORES)), trace=_trace
    )
    out = np.zeros((B, S, D), dtype=np.float32)
    for c in range(NCORES):
        b, g = c // GROUP, c % GROUP
        out[b, :, g * CW:(g + 1) * CW] = res.results[c]["outT"].T
    if _trace:
        return out, res
    return out



# revision 21
# speedup vs baseline: 1.3333x; 1.3333x over previous
"""Distributed causal multi-head attention for Trainium2 (8 NeuronCores).

Problem (hardcoded): x[2, 2048, 1024], 16 heads, head_dim 64, causal
softmax(QK^T/8)V then out-proj with bias. f32 in/out.

Sharding: data parallel on batch (cores 0-3 -> batch 0, 4-7 -> batch 1),
tensor parallel on heads within each group of 4 (4 heads per core).

Each core:
  - computes Q^T,K^T (head pairs packed to 128 partitions), V for its 4 heads
  - scores transposed S^T[k,q] = K Q^T so the softmax denominator comes out
    of the PE via an appended ones-column on V (no partition reductions)
  - exp without max-subtraction (scores are O(2), safe in fp32/bf16)
  - causal mask applied post-exp as a 0/1 bf16 multiply (DVE 4x mode);
    diagonal k-chunk pairs computed at reduced q-extent (512/256) to cut
    PE scores/ctx and ACT exp work ~12%
  - ROW-parallel out-proj: each core computes partial out[1024 oc, q] over
    its OWN 256 ctx rows (+ bo/4), then a ReduceScatter(add) over the
    4-core group scatters q-slices. Host reassembles q-slices per batch.
    This replaces the ctx AllGather: the collective is billed on its
    (4x smaller) output and RS0 overlaps pair-1 attention.

All matmuls bf16 (fp32 PSUM accumulation).
"""

import numpy as np
import ml_dtypes

from concourse import bass, bacc, mybir
from concourse import tile
from concourse.bass_utils import run_bass_kernel_spmd

BF16 = mybir.dt.bfloat16
F32 = mybir.dt.float32
Act = mybir.ActivationFunctionType

B, S, D = 2, 2048, 1024
H, HD = 16, 64
NCORES = 8
GROUP = 4            # cores per batch group
HPC = H // GROUP     # 4 heads per core
CW = HPC * HD        # 256 ctx rows per core
QC = 512             # q-chunk width
KC = 128             # k-chunk width
NQ = S // QC         # 4
NKC = S // KC        # 16
DCH = D // 128       # 8 contraction chunks of 128
OCT = D // 128       # 8 out-proj column tiles

_CACHE = {}


def _build_bass():
    nc = bacc.Bacc(
        "TRN2", target_bir_lowering=False, debug=False, num_devices=NCORES
    )
    _ccs = []
    _rds = []

    # per-core external inputs (same shapes on every core: SPMD)
    xT = nc.declare_dram_parameter("xT", [D, S], BF16, isOutput=False)
    wq = nc.declare_dram_parameter("wq", [D, CW], BF16, isOutput=False)
    wk = nc.declare_dram_parameter("wk", [D, CW], BF16, isOutput=False)
    wv = nc.declare_dram_parameter("wv", [D, CW], BF16, isOutput=False)
    wo = nc.declare_dram_parameter("wo", [CW, D], BF16, isOutput=False)
    boq = nc.declare_dram_parameter("boq", [128, OCT, 1], F32, isOutput=False)
    msk = nc.declare_dram_parameter("msk", [128, 4, QC], BF16, isOutput=False)
    vones = nc.declare_dram_parameter("vones", [128, NKC, HPC, 1], BF16, isOutput=False)
    selv = nc.declare_dram_parameter("selv", [1, 2, 128], BF16, isOutput=False)
    # ReduceScatter outputs: piece h covers q in [h*1024 + rank*256, +256)
    po_out = [nc.declare_dram_parameter(f"po{h}", [D, 2 * QC // GROUP], BF16,
                                        isOutput=True) for h in range(2)]

    with tile.TileContext(nc) as tc:
        with tc.tile_pool(name="dram", bufs=1, space="DRAM") as dram:
            rs_in = [dram.tile([GROUP, D, 2 * QC // GROUP], BF16, name=f"rs_in{h}")
                     for h in range(2)]
            rs_out = [dram.tile([D, 2 * QC // GROUP], BF16, name=f"rs_out{h}")
                      for h in range(2)]

            with tc.tile_pool(name="persist", bufs=1) as pp:
                wq_sb = pp.tile([128, DCH, CW], BF16, tag="wq_sb")
                wk_sb = pp.tile([128, DCH, CW], BF16, tag="wk_sb")
                wv_sb = pp.tile([128, DCH, CW], BF16, tag="wv_sb")
                wo_sb = pp.tile([128, CW // 128, D], BF16, tag="wo_sb")
                boq_sb = pp.tile([128, OCT, 1], F32, tag="boq_sb")
                msk_sb = pp.tile([128, 4, QC], BF16, tag="msk_sb")
                selv_sb = pp.tile([65, 2, 128], BF16, tag="selv_sb")
                qT_sb = pp.tile([128, 2, S], BF16, tag="qT_sb")
                kT_sb = pp.tile([128, 2, S], BF16, tag="kT_sb")
                v_aug = pp.tile([128, NKC, HPC, HD + 1], BF16, tag="v_aug")
                xT_sb = pp.tile([128, DCH, S], BF16, tag="xT_sb")

                # ---- input DMAs: few, large, ordered for earliest compute ----
                def ld(dst_ap, src_ap):
                    nc.sync.dma_start(dst_ap, src_ap)

                ld(wq_sb[:], wq.rearrange("(c p) w -> p c w", p=128))
                ld(xT_sb[:, :, 0:QC], xT[:, 0:QC].rearrange("(c p) q -> p c q", p=128))
                ld(wk_sb[:], wk.rearrange("(c p) w -> p c w", p=128))
                ld(xT_sb[:, :, QC:2 * QC],
                   xT[:, QC:2 * QC].rearrange("(c p) q -> p c q", p=128))
                ld(wv_sb[:], wv.rearrange("(c p) w -> p c w", p=128))
                ld(xT_sb[:, :, 2 * QC:3 * QC],
                   xT[:, 2 * QC:3 * QC].rearrange("(c p) q -> p c q", p=128))
                ld(msk_sb[:], msk[:])
                ld(v_aug[:, :, :, HD:HD + 1], vones[:])
                ld(selv_sb[64:65, :, :], selv[:])
                ld(xT_sb[:, :, 3 * QC:4 * QC],
                   xT[:, 3 * QC:4 * QC].rearrange("(c p) q -> p c q", p=128))
                ld(wo_sb[:], wo.rearrange("(c p) w -> p c w", p=128))
                ld(boq_sb[:], boq[:])

                with tc.tile_pool(name="sc_ps", bufs=2, space="PSUM") as scp, \
                     tc.tile_pool(name="ct_ps", bufs=2, space="PSUM") as ctp, \
                     tc.tile_pool(name="o_ps", bufs=2, space="PSUM") as outp, \
                     tc.tile_pool(name="es_pool", bufs=18) as esp, \
                     tc.tile_pool(name="stg_pool", bufs=6) as stgp, \
                     tc.tile_pool(name="ctxn_pool", bufs=6) as cxp, \
                     tc.tile_pool(name="po_pool", bufs=2) as pop, \
                     tc.tile_pool(name="nrm", bufs=3) as nrmp, \
                     tc.tile_pool(name="misc", bufs=1) as miscp:

                    ctxn = [[None] * NQ, [None] * NQ]

                    def proj_qk(pair, j):
                        # q in PSUM slot 0, k in slot 1 of one 2-bank tile
                        qs = slice(j * QC, (j + 1) * QC)
                        ps = scp.tile([128, 2, QC], F32, tag="st")
                        for i, (w_sb, dst) in enumerate(
                                ((wq_sb, qT_sb), (wk_sb, kT_sb))):
                            for c in range(DCH):
                                nc.tensor.matmul(
                                    ps[:, i, :],
                                    w_sb[:, c, pair * 128:(pair + 1) * 128],
                                    xT_sb[:, c, qs],
                                    start=(c == 0),
                                    stop=(c == DCH - 1),
                                )
                            nc.vector.tensor_copy(dst[:, pair, qs], ps[:, i, :])

                    def proj_v(pair, t0):
                        # 2 token-chunks per tile: one accum group per PSUM bank
                        ps = scp.tile([128, 2, QC], F32, tag="st")
                        for r in range(2):
                            t = t0 + r
                            for c in range(DCH):
                                nc.tensor.matmul(
                                    ps[:, r, 0:128],
                                    xT_sb[:, c, t * 128:(t + 1) * 128],
                                    wv_sb[:, c, pair * 128:(pair + 1) * 128],
                                    start=(c == 0),
                                    stop=(c == DCH - 1),
                                )
                        nc.vector.tensor_copy(
                            v_aug[:, t0:t0 + 2, 2 * pair:2 * pair + 2, 0:HD],
                            ps[:, :, 0:128].rearrange("p t (h w) -> p t h w", h=2),
                        )

                    def scores(h, j):
                        pair, hh = h // 2, h % 2
                        row = hh * 64
                        nkc = (j + 1) * 4
                        es_tiles = []
                        offs = []
                        for c0 in range(0, nkc, 2):
                            # diagonal pair (last two chunks) at q-extent 256
                            qa = 256 if c0 == nkc - 2 else 0
                            st = scp.tile([128, 2, QC], F32, tag="st")
                            for i in range(2):
                                c = c0 + i
                                nc.tensor.matmul(
                                    st[:, i, qa:],
                                    kT_sb[row:row + 64, pair, c * KC:(c + 1) * KC],
                                    qT_sb[row:row + 64, pair, j * QC + qa:(j + 1) * QC],
                                    start=True, stop=True,
                                )
                            es = esp.tile([128, 2, QC], BF16, tag="es")
                            nc.scalar.activation(es[:, :, qa:], st[:, :, qa:],
                                                 Act.Exp, scale=0.125)
                            if c0 >= nkc - 4:
                                m0 = c0 - (nkc - 4)
                                nc.vector.tensor_mul(
                                    es[:, :, qa:], es[:, :, qa:],
                                    msk_sb[:, m0:m0 + 2, qa:],
                                )
                            es_tiles.append(es)
                            offs.append(qa)
                        return es_tiles, offs

                    def ctx(h, j, es_tiles, offs):
                        nkc = (j + 1) * 4
                        ct = ctp.tile([HD + 1, QC], F32, tag="ct")
                        for c in range(nkc):
                            qa = offs[c // 2]
                            nc.tensor.matmul(
                                ct[:, qa:],
                                v_aug[:, c, h, :],
                                es_tiles[c // 2][:, c % 2, qa:],
                                start=(c == 0),
                                stop=(c == nkc - 1),
                            )
                        # ctx rows + den row in one copy (bf16 den: ~0.2% noise)
                        stg = stgp.tile([HD + 1, QC], BF16, tag="stg")
                        nc.vector.tensor_copy(stg[:], ct[:])
                        return stg

                    def attn2(pair, j):
                        # scores of head B run while head A's exp drains: no
                        # PE bubble waiting on the ACT->DVE chain
                        ha, hb = 2 * pair, 2 * pair + 1
                        esa, offa = scores(ha, j)
                        esb, offb = scores(hb, j)
                        sa = ctx(ha, j, esa, offa)
                        sb = ctx(hb, j, esb, offb)
                        norm(pair, j, sa, sb)

                    def norm(pair, j, stg_even, stg_odd):
                        bc = outp.tile([128, QC], F32, tag="ops")
                        nc.tensor.matmul(bc[:], selv_sb[64:65, 0, :],
                                         stg_even[HD:HD + 1, :],
                                         start=True, stop=False)
                        nc.tensor.matmul(bc[:], selv_sb[64:65, 1, :],
                                         stg_odd[HD:HD + 1, :],
                                         start=False, stop=True)
                        rb = nrmp.tile([64, 2, QC], BF16, tag="rb")
                        with nc.allow_low_precision(reason="1/den in bf16: ~0.2% fro"):
                            nc.vector.reciprocal(rb[:, 0, :], bc[0:64, :])
                            nc.vector.reciprocal(rb[:, 1, :], bc[64:128, :])
                        cx = cxp.tile([128, QC], BF16, tag="ctxn")
                        nc.vector.tensor_mul(cx[0:64, :], stg_even[0:HD, :], rb[:, 0, :])
                        nc.vector.tensor_mul(cx[64:128, :], stg_odd[0:HD, :], rb[:, 1, :])
                        ctxn[pair][j] = cx

                    def po(j):
                        posb = pop.tile([128, OCT, QC], BF16, tag="posb")
                        for o in range(OCT):
                            ps = outp.tile([128, QC], F32, tag="ops")
                            nc.tensor.matmul(ps[:], wo_sb[:, 0, o * 128:(o + 1) * 128],
                                             ctxn[0][j][:], start=True, stop=False)
                            nc.tensor.matmul(ps[:], wo_sb[:, 1, o * 128:(o + 1) * 128],
                                             ctxn[1][j][:], start=False, stop=True)
                            with nc.allow_low_precision(reason="bf16 RS partials"):
                                if o % 2 == 0:
                                    nc.vector.tensor_scalar_add(
                                        posb[:, o, :], ps[:], boq_sb[:, o, :])
                                else:
                                    nc.scalar.activation(
                                        posb[:, o, :], ps[:], Act.Identity,
                                        bias=boq_sb[:, o, :])
                        h, r0 = j // 2, (j % 2) * 2
                        for r in range(2):
                            nc.sync.dma_start(
                                rs_in[h][r0 + r, :, :].rearrange(
                                    "(o p) q -> p o q", p=128),
                                posb[:, :, r * 256:(r + 1) * 256],
                            )

                    def rs(h):
                        _ccs.append(nc.gpsimd.collective_compute(
                            "ReduceScatter",
                            mybir.AluOpType.add,
                            replica_groups=[[0, 1, 2, 3], [4, 5, 6, 7]],
                            ins=[rs_in[h][:].opt()],
                            outs=[rs_out[h][:].opt()],
                        ))
                        _rds.append((nc.sync.dma_start(po_out[h][:], rs_out[h][:]), h))

                    # ---- schedule: j-outer, pairs interleaved, projections
                    # spread through the stream as PE filler so exp (ACT)
                    # starts early and po(j)/RS fire as soon as possible ----
                    for j in range(NQ):
                        proj_qk(0, j)
                        proj_v(0, 4 * j)
                        proj_v(0, 4 * j + 2)
                        attn2(0, j)
                        proj_qk(1, j)
                        proj_v(1, 4 * j)
                        proj_v(1, 4 * j + 2)
                        attn2(1, j)
                        po(j)
                        if j == 1:
                            rs(0)
                    rs(1)

    upd = _ccs[0].ins.sync_info.on_update[0]
    cc_done_sem = bass.SemaphoreHandle(upd.ant_name, upd.id)
    for rd, h in _rds:
        rd.wait_op(cc_done_sem, h + 1, "sem-ge", check=False)
    nc.compile()
    return nc


def _causal_mask():
    # msk[kp, m, qf] = 1 where (m*128 + kp) <= qf else 0  (keep k <= q)
    kp = np.arange(128)[:, None, None]
    m = np.arange(4)[None, :, None]
    qf = np.arange(QC)[None, None, :]
    return (m * 128 + kp <= qf).astype(ml_dtypes.bfloat16)


def _in_maps(x, Wq, Wk, Wv, Wo, bo):
    bf = ml_dtypes.bfloat16
    msk = _causal_mask()
    selv = np.zeros((1, 2, 128), dtype=bf)
    selv[0, 0, 0:64] = 1.0
    selv[0, 1, 64:128] = 1.0
    boq = (bo.reshape(OCT, 128).T / GROUP).astype(np.float32)[:, :, None]
    xT = [np.ascontiguousarray(x[b].T).astype(bf) for b in range(B)]
    maps = []
    for c in range(NCORES):
        b, g = c // GROUP, c % GROUP
        cs = slice(g * CW, (g + 1) * CW)
        maps.append({
            "xT": xT[b],
            "wq": np.ascontiguousarray(Wq[:, cs]).astype(bf),
            "wk": np.ascontiguousarray(Wk[:, cs]).astype(bf),
            "wv": np.ascontiguousarray(Wv[:, cs]).astype(bf),
            "wo": np.ascontiguousarray(Wo[cs, :]).astype(bf),
            "boq": boq,
            "msk": msk,
            "vones": np.ones((128, NKC, HPC, 1), dtype=bf),
            "selv": selv,
        })
    return maps


def kernel(x, Wq, Wk, Wv, Wo, bo, _trace=False):
    x = np.asarray(x, dtype=np.float32)
    Wq, Wk, Wv, Wo, bo = (np.asarray(a, dtype=np.float32) for a in (Wq, Wk, Wv, Wo, bo))
    if "nc" not in _CACHE:
        _CACHE["nc"] = _build_bass()
    nc = _CACHE["nc"]
    res = run_bass_kernel_spmd(
        nc, _in_maps(x, Wq, Wk, Wv, Wo, bo), list(range(NCORES)), trace=_trace
    )
    out = np.zeros((B, S, D), dtype=np.float32)
    PW = 2 * QC // GROUP  # 256-wide q pieces
    for c in range(NCORES):
        b, g = c // GROUP, c % GROUP
        for h in range(2):
            piece = np.asarray(res.results[c][f"po{h}"]).astype(np.float32)
            q0 = h * 2 * QC + g * PW
            out[b, q0:q0 + PW, :] = piece.T
    if _trace:
        return out, res
    return out


# revision 28
# speedup vs baseline: 1.4187x; 1.0641x over previous
"""Distributed causal multi-head attention for Trainium2 (8 NeuronCores).

Problem (hardcoded): x[2, 2048, 1024], 16 heads, head_dim 64, causal
softmax(QK^T/8)V then out-proj with bias. f32 in/out.

Sharding: data parallel on batch (cores 0-3 -> batch 0, 4-7 -> batch 1),
tensor parallel on heads within each group of 4 (4 heads per core).

Each core:
  - computes Q^T,K^T (head pairs packed to 128 partitions), V for its 4 heads
  - scores transposed S^T[k,q] = K Q^T so the softmax denominator comes out
    of the PE via an appended ones-column on V (no partition reductions)
  - exp without max-subtraction (scores are O(2), safe in fp32/bf16)
  - causal mask applied post-exp as a 0/1 bf16 multiply (DVE 4x mode);
    diagonal k-chunk pairs computed at reduced q-extent (512/256) to cut
    PE scores/ctx and ACT exp work ~12%
  - ROW-parallel out-proj: each core computes partial out[1024 oc, q] over
    its OWN 256 ctx rows (+ bo/4), then a ReduceScatter(add) over the
    4-core group scatters q-slices. Host reassembles q-slices per batch.
    This replaces the ctx AllGather: the collective is billed on its
    (4x smaller) output and RS0 overlaps pair-1 attention.

All matmuls bf16 (fp32 PSUM accumulation).
"""

import numpy as np
import ml_dtypes

from concourse import bass, bacc, mybir
from concourse import tile
from concourse.bass_utils import run_bass_kernel_spmd

BF16 = mybir.dt.bfloat16
F32 = mybir.dt.float32
Act = mybir.ActivationFunctionType

B, S, D = 2, 2048, 1024
H, HD = 16, 64
NCORES = 8
GROUP = 4            # cores per batch group
HPC = H // GROUP     # 4 heads per core
CW = HPC * HD        # 256 ctx rows per core
QC = 512             # q-chunk width
KC = 128             # k-chunk width
NQ = S // QC         # 4
NKC = S // KC        # 16
DCH = D // 128       # 8 contraction chunks of 128
OCT = D // 128       # 8 out-proj column tiles
PWA = 384            # RS region A (q<1536) piece width per rank
PWB = 128            # RS region B (q>=1536) piece width per rank

_CACHE = {}


def _build_bass():
    nc = bacc.Bacc(
        "TRN2", target_bir_lowering=False, debug=False, num_devices=NCORES
    )
    _ccs = []
    _rds = []

    # per-core external inputs (same shapes on every core: SPMD)
    xT = nc.declare_dram_parameter("xT", [D, S], BF16, isOutput=False)
    wq = nc.declare_dram_parameter("wq", [D, CW], BF16, isOutput=False)
    wk = nc.declare_dram_parameter("wk", [D, CW], BF16, isOutput=False)
    wv = nc.declare_dram_parameter("wv", [D, CW], BF16, isOutput=False)
    wo = nc.declare_dram_parameter("wo", [CW, D], BF16, isOutput=False)
    boq = nc.declare_dram_parameter("boq", [128, OCT, 1], F32, isOutput=False)
    msk = nc.declare_dram_parameter("msk", [128, 4, QC], BF16, isOutput=False)
    vones = nc.declare_dram_parameter("vones", [128, NKC, HPC, 1], BF16, isOutput=False)
    selv = nc.declare_dram_parameter("selv", [1, 2, 128], BF16, isOutput=False)
    # ReduceScatter piece outputs: region A q = rank*384 + [0,384),
    # region B q = 1536 + rank*128 + [0,128)
    po_out = [nc.declare_dram_parameter(f"po{h}", [D, w], BF16, isOutput=True)
              for h, w in ((0, 384), (1, 128))]

    with tile.TileContext(nc) as tc:
        with tc.tile_pool(name="dram", bufs=1, space="DRAM") as dram:
            rs_in = [dram.tile([GROUP, D, w], BF16, name=f"rs_in{h}")
                     for h, w in ((0, PWA), (1, PWB))]
            rs_out = [dram.tile([D, w], BF16, name=f"rs_out{h}")
                      for h, w in ((0, PWA), (1, PWB))]

            with tc.tile_pool(name="persist", bufs=1) as pp:
                wq_sb = pp.tile([128, DCH, CW], BF16, tag="wq_sb")
                wk_sb = pp.tile([128, DCH, CW], BF16, tag="wk_sb")
                wv_sb = pp.tile([128, DCH, CW], BF16, tag="wv_sb")
                wo_sb = pp.tile([128, CW // 128, D], BF16, tag="wo_sb")
                boq_sb = pp.tile([128, OCT, 1], F32, tag="boq_sb")
                msk_sb = pp.tile([128, 4, QC], BF16, tag="msk_sb")
                selv_sb = pp.tile([65, 2, 128], BF16, tag="selv_sb")
                qT_sb = pp.tile([128, 2, S], BF16, tag="qT_sb")
                kT_sb = pp.tile([128, 2, S], BF16, tag="kT_sb")
                v_aug = pp.tile([128, NKC, HPC, HD + 1], BF16, tag="v_aug")
                xT_sb = pp.tile([128, DCH, S], BF16, tag="xT_sb")

                # ---- input DMAs: few, large, ordered for earliest compute ----
                def ld(dst_ap, src_ap):
                    nc.sync.dma_start(dst_ap, src_ap)

                ld(wq_sb[:], wq.rearrange("(c p) w -> p c w", p=128))
                ld(xT_sb[:, :, 0:QC], xT[:, 0:QC].rearrange("(c p) q -> p c q", p=128))
                ld(wk_sb[:], wk.rearrange("(c p) w -> p c w", p=128))
                ld(xT_sb[:, :, QC:2 * QC],
                   xT[:, QC:2 * QC].rearrange("(c p) q -> p c q", p=128))
                ld(wv_sb[:], wv.rearrange("(c p) w -> p c w", p=128))
                ld(xT_sb[:, :, 2 * QC:3 * QC],
                   xT[:, 2 * QC:3 * QC].rearrange("(c p) q -> p c q", p=128))
                ld(msk_sb[:], msk[:])
                ld(v_aug[:, :, :, HD:HD + 1], vones[:])
                ld(selv_sb[64:65, :, :], selv[:])
                ld(xT_sb[:, :, 3 * QC:4 * QC],
                   xT[:, 3 * QC:4 * QC].rearrange("(c p) q -> p c q", p=128))
                ld(wo_sb[:], wo.rearrange("(c p) w -> p c w", p=128))
                ld(boq_sb[:], boq[:])

                with tc.tile_pool(name="sc_ps", bufs=2, space="PSUM") as scp, \
                     tc.tile_pool(name="ct_ps", bufs=2, space="PSUM") as ctp, \
                     tc.tile_pool(name="o_ps", bufs=2, space="PSUM") as outp, \
                     tc.tile_pool(name="es_pool", bufs=18) as esp, \
                     tc.tile_pool(name="stg_pool", bufs=8) as stgp, \
                     tc.tile_pool(name="ctxn_pool", bufs=8) as cxp, \
                     tc.tile_pool(name="po_pool", bufs=2) as pop, \
                     tc.tile_pool(name="nrm", bufs=3) as nrmp, \
                     tc.tile_pool(name="misc", bufs=1) as miscp:

                    ctxn = [[None] * NQ, [None] * NQ]

                    # ---- PE filler queue: self-contained items (alloc, use
                    # and release one outp PSUM bank each) popped between
                    # attention chunk iterations so PE never idles while ACT
                    # digests exps, and ACT never idles during projections ----
                    fill_hi = []   # po items: drained first (unblock the RS)
                    fill_lo = []   # proj items: safe to run early
                    _fp = [0, 0]

                    def pop_fill(budget):
                        spent = 0.0
                        while spent < budget:
                            if _fp[0] < len(fill_hi):
                                cost, fn = fill_hi[_fp[0]]
                                _fp[0] += 1
                            elif _fp[1] < len(fill_lo):
                                cost, fn = fill_lo[_fp[1]]
                                _fp[1] += 1
                            else:
                                return
                            fn()
                            spent += cost

                    def drain_lo(idx):
                        while _fp[1] < idx:
                            cost, fn = fill_lo[_fp[1]]
                            _fp[1] += 1
                            fn()

                    def drain_hi():
                        while _fp[0] < len(fill_hi):
                            cost, fn = fill_hi[_fp[0]]
                            _fp[0] += 1
                            fn()

                    def proj_qk_item(pair, j, w_sb, dst):
                        def run():
                            qs = slice(j * QC, (j + 1) * QC)
                            ps = outp.tile([128, QC], F32, tag="ops")
                            for c in range(DCH):
                                nc.tensor.matmul(
                                    ps[:],
                                    w_sb[:, c, pair * 128:(pair + 1) * 128],
                                    xT_sb[:, c, qs],
                                    start=(c == 0),
                                    stop=(c == DCH - 1),
                                )
                            nc.vector.tensor_copy(dst[:, pair, qs], ps[:])
                        return (1.9, run)

                    def proj_v_item(pair, t):
                        def run():
                            ps = outp.tile([128, QC], F32, tag="ops")
                            for c in range(DCH):
                                nc.tensor.matmul(
                                    ps[:, 0:128],
                                    xT_sb[:, c, t * 128:(t + 1) * 128],
                                    wv_sb[:, c, pair * 128:(pair + 1) * 128],
                                    start=(c == 0),
                                    stop=(c == DCH - 1),
                                )
                            nc.vector.tensor_copy(
                                v_aug[:, t, 2 * pair:2 * pair + 2, 0:HD],
                                ps[:, 0:128].rearrange("p (h w) -> p h w", h=2),
                            )
                        return (0.8, run)

                    def attn2(pair, j):
                        # chunk-level software pipeline across the head pair:
                        # PE emits scores(A,i), scores(B,i), then ctx for
                        # chunk-pair i-1 of both heads, so the PE never parks
                        # on the scores->exp->mask chain and ACT stays fed.
                        ha, hb = 2 * pair, 2 * pair + 1
                        nkc = (j + 1) * 4
                        npr = nkc // 2
                        es = {ha: [], hb: []}
                        offs = []

                        def scores_pair(h, i, qa):
                            row = (h % 2) * 64
                            st = scp.tile([128, 2, QC], F32, tag="st")
                            for k in range(2):
                                c = 2 * i + k
                                nc.tensor.matmul(
                                    st[:, k, qa:],
                                    kT_sb[row:row + 64, pair, c * KC:(c + 1) * KC],
                                    qT_sb[row:row + 64, pair, j * QC + qa:(j + 1) * QC],
                                    start=True, stop=True,
                                )
                            e = esp.tile([128, 2, QC], BF16, tag="es")
                            nc.scalar.activation(e[:, :, qa:], st[:, :, qa:],
                                                 Act.Exp, scale=0.125)
                            if i >= npr - 2:
                                m0 = 2 * i - (nkc - 4)
                                nc.vector.tensor_mul(
                                    e[:, :, qa:], e[:, :, qa:],
                                    msk_sb[:, m0:m0 + 2, qa:],
                                )
                            es[h].append(e)

                        def ctx_pair(ct, h, i):
                            qa = offs[i]
                            for k in range(2):
                                c = 2 * i + k
                                nc.tensor.matmul(
                                    ct[:, qa:],
                                    v_aug[:, c, h, :],
                                    es[h][i][:, k, qa:],
                                    start=(c == 0),
                                    stop=(c == nkc - 1),
                                )

                        ctA = ctp.tile([HD + 1, QC], F32, tag="ct")
                        ctB = ctp.tile([HD + 1, QC], F32, tag="ct")
                        for i in range(npr):
                            qa = 256 if i == npr - 1 else 0
                            offs.append(qa)
                            scores_pair(ha, i, qa)
                            scores_pair(hb, i, qa)
                            if i >= 1:
                                ctx_pair(ctA, ha, i - 1)
                                ctx_pair(ctB, hb, i - 1)
                            pop_fill(1.8)
                        ctx_pair(ctA, ha, npr - 1)
                        ctx_pair(ctB, hb, npr - 1)
                        # ctx rows + den row in one copy (bf16 den: ~0.2% noise)
                        sa = stgp.tile([HD + 1, QC], BF16, tag="stg")
                        nc.vector.tensor_copy(sa[:], ctA[:])
                        sb = stgp.tile([HD + 1, QC], BF16, tag="stg")
                        nc.vector.tensor_copy(sb[:], ctB[:])
                        norm(pair, j, sa, sb)

                    def norm(pair, j, stg_even, stg_odd):
                        bc = outp.tile([128, QC], F32, tag="ops")
                        nc.tensor.matmul(bc[:], selv_sb[64:65, 0, :],
                                         stg_even[HD:HD + 1, :],
                                         start=True, stop=False)
                        nc.tensor.matmul(bc[:], selv_sb[64:65, 1, :],
                                         stg_odd[HD:HD + 1, :],
                                         start=False, stop=True)
                        rb = nrmp.tile([64, 2, QC], BF16, tag="rb")
                        with nc.allow_low_precision(reason="1/den in bf16: ~0.2% fro"):
                            nc.vector.reciprocal(rb[:, 0, :], bc[0:64, :])
                            nc.vector.reciprocal(rb[:, 1, :], bc[64:128, :])
                        cx = cxp.tile([128, QC], BF16, tag="ctxn")
                        nc.vector.tensor_mul(cx[0:64, :], stg_even[0:HD, :], rb[:, 0, :])
                        nc.vector.tensor_mul(cx[64:128, :], stg_odd[0:HD, :], rb[:, 1, :])
                        ctxn[pair][j] = cx

                    def po_items(j):
                        posb = pop.tile([128, OCT, QC], BF16, tag="posb")

                        def mk_o(o):
                            def run():
                                ps = outp.tile([128, QC], F32, tag="ops")
                                nc.tensor.matmul(
                                    ps[:], wo_sb[:, 0, o * 128:(o + 1) * 128],
                                    ctxn[0][j][:], start=True, stop=False)
                                nc.tensor.matmul(
                                    ps[:], wo_sb[:, 1, o * 128:(o + 1) * 128],
                                    ctxn[1][j][:], start=False, stop=True)
                                with nc.allow_low_precision(reason="bf16 RS partials"):
                                    nc.vector.tensor_scalar_add(
                                        posb[:, o, :], ps[:], boq_sb[:, o, :])
                            return (0.6, run)

                        def dma_run():
                            # scatter into the straddling RS slabs:
                            # region A (q<1536): 4 x 384; region B: 4 x 128
                            q0, qe = j * QC, (j + 1) * QC
                            q = q0
                            while q < qe:
                                if q < 3 * QC:
                                    slab = q // PWA
                                    end = min((slab + 1) * PWA, 3 * QC, qe)
                                    dst = rs_in[0][slab, :, q - slab * PWA:
                                                   q - slab * PWA + end - q]
                                else:
                                    slab = (q - 3 * QC) // PWB
                                    end = min(3 * QC + (slab + 1) * PWB, qe)
                                    dst = rs_in[1][slab, :, q - 3 * QC - slab * PWB:
                                                   q - 3 * QC - slab * PWB + end - q]
                                nc.sync.dma_start(
                                    dst.rearrange("(o p) q -> p o q", p=128),
                                    posb[:, :, q - q0:end - q0],
                                )
                                q = end

                        for o in range(OCT):
                            fill_hi.append(mk_o(o))
                        fill_hi.append((0.1, dma_run))

                    def rs(h):
                        _ccs.append(nc.gpsimd.collective_compute(
                            "ReduceScatter",
                            mybir.AluOpType.add,
                            replica_groups=[[0, 1, 2, 3], [4, 5, 6, 7]],
                            ins=[rs_in[h][:].opt()],
                            outs=[rs_out[h][:].opt()],
                        ))
                        _rds.append((nc.sync.dma_start(po_out[h][:], rs_out[h][:]), h))

                    # ---- schedule: j-outer, pairs interleaved; projections
                    # and out-proj partials drain through the filler queue so
                    # PE and ACT overlap throughout; RS{j0..j2} fires right
                    # after po(2), RS{j3} is the (smallest possible) tail ----
                    marks = {}
                    for j in range(NQ):
                        for pair in (0, 1):
                            fill_lo.append(proj_qk_item(pair, j, wq_sb, qT_sb))
                            fill_lo.append(proj_qk_item(pair, j, wk_sb, kT_sb))
                            for t in range(4 * j, 4 * j + 4):
                                fill_lo.append(proj_v_item(pair, t))
                            marks[(pair, j)] = len(fill_lo)
                    for j in range(NQ):
                        for pair in (0, 1):
                            drain_lo(marks[(pair, j)])
                            attn2(pair, j)
                        po_items(j)
                        if j == 2:
                            drain_hi()
                            rs(0)
                    drain_hi()
                    drain_lo(len(fill_lo))
                    rs(1)

    upd = _ccs[0].ins.sync_info.on_update[0]
    cc_done_sem = bass.SemaphoreHandle(upd.ant_name, upd.id)
    for rd, h in _rds:
        rd.wait_op(cc_done_sem, h + 1, "sem-ge", check=False)
    nc.compile()
    return nc


def _causal_mask():
    # msk[kp, m, qf] = 1 where (m*128 + kp) <= qf else 0  (keep k <= q)
    kp = np.arange(128)[:, None, None]
    m = np.arange(4)[None, :, None]
    qf = np.arange(QC)[None, None, :]
    return (m * 128 + kp <= qf).astype(ml_dtypes.bfloat16)


def _in_maps(x, Wq, Wk, Wv, Wo, bo):
    bf = ml_dtypes.bfloat16
    msk = _causal_mask()
    selv = np.zeros((1, 2, 128), dtype=bf)
    selv[0, 0, 0:64] = 1.0
    selv[0, 1, 64:128] = 1.0
    boq = (bo.reshape(OCT, 128).T / GROUP).astype(np.float32)[:, :, None]
    xT = [np.ascontiguousarray(x[b].T).astype(bf) for b in range(B)]
    maps = []
    for c in range(NCORES):
        b, g = c // GROUP, c % GROUP
        cs = slice(g * CW, (g + 1) * CW)
        maps.append({
            "xT": xT[b],
            "wq": np.ascontiguousarray(Wq[:, cs]).astype(bf),
            "wk": np.ascontiguousarray(Wk[:, cs]).astype(bf),
            "wv": np.ascontiguousarray(Wv[:, cs]).astype(bf),
            "wo": np.ascontiguousarray(Wo[cs, :]).astype(bf),
            "boq": boq,
            "msk": msk,
            "vones": np.ones((128, NKC, HPC, 1), dtype=bf),
            "selv": selv,
        })
    return maps


def kernel(x, Wq, Wk, Wv, Wo, bo, _trace=False):
    x = np.asarray(x, dtype=np.float32)
    Wq, Wk, Wv, Wo, bo = (np.asarray(a, dtype=np.float32) for a in (Wq, Wk, Wv, Wo, bo))
    if "nc" not in _CACHE:
        _CACHE["nc"] = _build_bass()
    nc = _CACHE["nc"]
    res = run_bass_kernel_spmd(
        nc, _in_maps(x, Wq, Wk, Wv, Wo, bo), list(range(NCORES)), trace=_trace
    )
    out = np.zeros((B, S, D), dtype=np.float32)
    for c in range(NCORES):
        b, g = c // GROUP, c % GROUP
        pa = np.asarray(res.results[c]["po0"]).astype(np.float32)
        out[b, g * PWA:(g + 1) * PWA, :] = pa.T
        pb = np.asarray(res.results[c]["po1"]).astype(np.float32)
        q0 = 3 * QC + g * PWB
        out[b, q0:q0 + PWB, :] = pb.T
    if _trace:
        return out, res
    return out


# revision 30
# speedup vs baseline: 1.4671x; 1.0341x over previous
"""Distributed causal multi-head attention for Trainium2 (8 NeuronCores).

Problem (hardcoded): x[2, 2048, 1024], 16 heads, head_dim 64, causal
softmax(QK^T/8)V then out-proj with bias. f32 in/out.

Sharding: data parallel on batch (cores 0-3 -> batch 0, 4-7 -> batch 1),
tensor parallel on heads within each group of 4 (4 heads per core).

Each core:
  - computes Q^T,K^T (head pairs packed to 128 partitions), V for its 4 heads
  - scores transposed S^T[k,q] = K Q^T so the softmax denominator comes out
    of the PE via an appended ones-column on V (no partition reductions)
  - exp without max-subtraction (scores are O(2), safe in fp32/bf16)
  - causal mask applied post-exp as a 0/1 bf16 multiply (DVE 4x mode);
    diagonal k-chunk pairs computed at reduced q-extent (512/256) to cut
    PE scores/ctx and ACT exp work ~12%
  - ROW-parallel out-proj: each core computes partial out[1024 oc, q] over
    its OWN 256 ctx rows (+ bo/4), then a ReduceScatter(add) over the
    4-core group scatters q-slices. Host reassembles q-slices per batch.
    This replaces the ctx AllGather: the collective is billed on its
    (4x smaller) output and RS0 overlaps pair-1 attention.

All matmuls bf16 (fp32 PSUM accumulation).
"""

import numpy as np
import ml_dtypes

from concourse import bass, bacc, mybir
from concourse import tile
from concourse.bass_utils import run_bass_kernel_spmd

BF16 = mybir.dt.bfloat16
F32 = mybir.dt.float32
Act = mybir.ActivationFunctionType

B, S, D = 2, 2048, 1024
H, HD = 16, 64
NCORES = 8
GROUP = 4            # cores per batch group
HPC = H // GROUP     # 4 heads per core
CW = HPC * HD        # 256 ctx rows per core
QC = 512             # q-chunk width
KC = 128             # k-chunk width
NQ = S // QC         # 4
NKC = S // KC        # 16
DCH = D // 128       # 8 contraction chunks of 128
OCT = D // 128       # 8 out-proj column tiles
PWA = 384            # RS region A (q<1536) piece width per rank
PWB = 128            # RS region B (q>=1536) piece width per rank

_CACHE = {}


def _build_bass():
    nc = bacc.Bacc(
        "TRN2", target_bir_lowering=False, debug=False, num_devices=NCORES
    )
    _ccs = []
    _rds = []

    # per-core external inputs (same shapes on every core: SPMD)
    xT = nc.declare_dram_parameter("xT", [D, S], BF16, isOutput=False)
    wq = nc.declare_dram_parameter("wq", [D, CW], BF16, isOutput=False)
    wk = nc.declare_dram_parameter("wk", [D, CW], BF16, isOutput=False)
    wv = nc.declare_dram_parameter("wv", [D, CW], BF16, isOutput=False)
    wo = nc.declare_dram_parameter("wo", [CW, D], BF16, isOutput=False)
    boq = nc.declare_dram_parameter("boq", [128, OCT, 1], F32, isOutput=False)
    msk = nc.declare_dram_parameter("msk", [128, 4, QC], BF16, isOutput=False)
    vones = nc.declare_dram_parameter("vones", [128, NKC, HPC, 1], BF16, isOutput=False)
    selv = nc.declare_dram_parameter("selv", [1, 2, 128], BF16, isOutput=False)
    # ReduceScatter piece outputs: region A q = rank*384 + [0,384),
    # region B q = 1536 + rank*128 + [0,128)
    po_out = [nc.declare_dram_parameter(f"po{h}", [D, w], BF16, isOutput=True)
              for h, w in ((0, 384), (1, 128))]

    with tile.TileContext(nc) as tc:
        with tc.tile_pool(name="dram", bufs=1, space="DRAM") as dram:
            rs_in = [dram.tile([GROUP, D, w], BF16, name=f"rs_in{h}")
                     for h, w in ((0, PWA), (1, PWB))]
            rs_out = [dram.tile([D, w], BF16, name=f"rs_out{h}")
                      for h, w in ((0, PWA), (1, PWB))]

            with tc.tile_pool(name="persist", bufs=1) as pp:
                wq_sb = pp.tile([128, DCH, CW], BF16, tag="wq_sb")
                wk_sb = pp.tile([128, DCH, CW], BF16, tag="wk_sb")
                wv_sb = pp.tile([128, DCH, CW], BF16, tag="wv_sb")
                wo_sb = pp.tile([128, CW // 128, D], BF16, tag="wo_sb")
                boq_sb = pp.tile([128, OCT, 1], F32, tag="boq_sb")
                msk_sb = pp.tile([128, 4, QC], BF16, tag="msk_sb")
                selv_sb = pp.tile([65, 2, 128], BF16, tag="selv_sb")
                qT_sb = pp.tile([128, 2, S], BF16, tag="qT_sb")
                kT_sb = pp.tile([128, 2, S], BF16, tag="kT_sb")
                v_aug = pp.tile([128, NKC, HPC, HD + 1], BF16, tag="v_aug")
                xT_sb = pp.tile([128, DCH, S], BF16, tag="xT_sb")

                # ---- input DMAs: few, large, ordered for earliest compute ----
                def ld(dst_ap, src_ap):
                    nc.sync.dma_start(dst_ap, src_ap)

                ld(wq_sb[:], wq.rearrange("(c p) w -> p c w", p=128))
                ld(xT_sb[:, :, 0:QC], xT[:, 0:QC].rearrange("(c p) q -> p c q", p=128))
                ld(wk_sb[:], wk.rearrange("(c p) w -> p c w", p=128))
                ld(xT_sb[:, :, QC:2 * QC],
                   xT[:, QC:2 * QC].rearrange("(c p) q -> p c q", p=128))
                ld(wv_sb[:], wv.rearrange("(c p) w -> p c w", p=128))
                ld(xT_sb[:, :, 2 * QC:3 * QC],
                   xT[:, 2 * QC:3 * QC].rearrange("(c p) q -> p c q", p=128))
                ld(msk_sb[:], msk[:])
                ld(v_aug[:, :, :, HD:HD + 1], vones[:])
                ld(selv_sb[64:65, :, :], selv[:])
                ld(xT_sb[:, :, 3 * QC:4 * QC],
                   xT[:, 3 * QC:4 * QC].rearrange("(c p) q -> p c q", p=128))
                ld(wo_sb[:], wo.rearrange("(c p) w -> p c w", p=128))
                ld(boq_sb[:], boq[:])

                with tc.tile_pool(name="sc_ps", bufs=2, space="PSUM") as scp, \
                     tc.tile_pool(name="ct_ps", bufs=2, space="PSUM") as ctp, \
                     tc.tile_pool(name="o_ps", bufs=2, space="PSUM") as outp, \
                     tc.tile_pool(name="es_pool", bufs=18) as esp, \
                     tc.tile_pool(name="stg_pool", bufs=8) as stgp, \
                     tc.tile_pool(name="ctxn_pool", bufs=8) as cxp, \
                     tc.tile_pool(name="po_pool", bufs=2) as pop, \
                     tc.tile_pool(name="nrm", bufs=3) as nrmp, \
                     tc.tile_pool(name="misc", bufs=1) as miscp:

                    ctxn = [[None] * NQ, [None] * NQ]

                    # ---- PE filler queue: self-contained items (alloc, use
                    # and release one outp PSUM bank each) popped between
                    # attention chunk iterations so PE never idles while ACT
                    # digests exps, and ACT never idles during projections ----
                    fill_hi = []   # po items: drained first (unblock the RS)
                    fill_lo = []   # proj items: safe to run early
                    _fp = [0, 0]

                    def pop_fill(budget):
                        spent = 0.0
                        while spent < budget:
                            if _fp[0] < len(fill_hi):
                                cost, fn = fill_hi[_fp[0]]
                                _fp[0] += 1
                            elif _fp[1] < len(fill_lo):
                                cost, fn = fill_lo[_fp[1]]
                                _fp[1] += 1
                            else:
                                return
                            fn()
                            spent += cost

                    def drain_lo(idx):
                        while _fp[1] < idx:
                            cost, fn = fill_lo[_fp[1]]
                            _fp[1] += 1
                            fn()

                    def drain_hi():
                        while _fp[0] < len(fill_hi):
                            cost, fn = fill_hi[_fp[0]]
                            _fp[0] += 1
                            fn()

                    def proj_qk_item(pair, j, w_sb, dst):
                        def run():
                            qs = slice(j * QC, (j + 1) * QC)
                            ps = outp.tile([128, QC], F32, tag="ops")
                            for c in range(DCH):
                                nc.tensor.matmul(
                                    ps[:],
                                    w_sb[:, c, pair * 128:(pair + 1) * 128],
                                    xT_sb[:, c, qs],
                                    start=(c == 0),
                                    stop=(c == DCH - 1),
                                )
                            nc.vector.tensor_copy(dst[:, pair, qs], ps[:])
                        return (1.9, run)

                    def proj_v_item(pair, t):
                        def run():
                            ps = outp.tile([128, QC], F32, tag="ops")
                            for c in range(DCH):
                                nc.tensor.matmul(
                                    ps[:, 0:128],
                                    xT_sb[:, c, t * 128:(t + 1) * 128],
                                    wv_sb[:, c, pair * 128:(pair + 1) * 128],
                                    start=(c == 0),
                                    stop=(c == DCH - 1),
                                )
                            nc.vector.tensor_copy(
                                v_aug[:, t, 2 * pair:2 * pair + 2, 0:HD],
                                ps[:, 0:128].rearrange("p (h w) -> p h w", h=2),
                            )
                        return (0.8, run)

                    def attn2(pair, j, mid_mark=None):
                        mid_i = max(0, 2 * j - 2)
                        # chunk-level software pipeline across the head pair:
                        # PE emits scores(A,i), scores(B,i), then ctx for
                        # chunk-pair i-1 of both heads, so the PE never parks
                        # on the scores->exp->mask chain and ACT stays fed.
                        ha, hb = 2 * pair, 2 * pair + 1
                        nkc = (j + 1) * 4
                        npr = nkc // 2
                        es = {ha: [], hb: []}
                        offs = []

                        def scores_pair(h, i, qa):
                            row = (h % 2) * 64
                            st = scp.tile([128, 2, QC], F32, tag="st")
                            for k in range(2):
                                c = 2 * i + k
                                nc.tensor.matmul(
                                    st[:, k, qa:],
                                    kT_sb[row:row + 64, pair, c * KC:(c + 1) * KC],
                                    qT_sb[row:row + 64, pair, j * QC + qa:(j + 1) * QC],
                                    start=True, stop=True,
                                )
                            e = esp.tile([128, 2, QC], BF16, tag="es")
                            nc.scalar.activation(e[:, :, qa:], st[:, :, qa:],
                                                 Act.Exp, scale=0.125)
                            if i >= npr - 2:
                                m0 = 2 * i - (nkc - 4)
                                nc.vector.tensor_mul(
                                    e[:, :, qa:], e[:, :, qa:],
                                    msk_sb[:, m0:m0 + 2, qa:],
                                )
                            es[h].append(e)

                        def ctx_pair(ct, h, i):
                            qa = offs[i]
                            for k in range(2):
                                c = 2 * i + k
                                nc.tensor.matmul(
                                    ct[:, qa:],
                                    v_aug[:, c, h, :],
                                    es[h][i][:, k, qa:],
                                    start=(c == 0),
                                    stop=(c == nkc - 1),
                                )

                        ctA = ctp.tile([HD + 1, QC], F32, tag="ct")
                        ctB = ctp.tile([HD + 1, QC], F32, tag="ct")
                        for i in range(npr):
                            qa = 256 if i == npr - 1 else 0
                            offs.append(qa)
                            scores_pair(ha, i, qa)
                            scores_pair(hb, i, qa)
                            if i >= 2:
                                ctx_pair(ctA, ha, i - 2)
                                ctx_pair(ctB, hb, i - 2)
                            if i == mid_i and mid_mark is not None:
                                drain_lo(mid_mark)
                            pop_fill(0.8)
                        ctx_pair(ctA, ha, npr - 2)
                        ctx_pair(ctB, hb, npr - 2)
                        ctx_pair(ctA, ha, npr - 1)
                        ctx_pair(ctB, hb, npr - 1)
                        # ctx rows + den row in one copy (bf16 den: ~0.2% noise)
                        sa = stgp.tile([HD + 1, QC], BF16, tag="stg")
                        nc.vector.tensor_copy(sa[:], ctA[:])
                        sb = stgp.tile([HD + 1, QC], BF16, tag="stg")
                        nc.vector.tensor_copy(sb[:], ctB[:])
                        norm(pair, j, sa, sb)

                    def norm(pair, j, stg_even, stg_odd):
                        bc = outp.tile([128, QC], F32, tag="ops")
                        nc.tensor.matmul(bc[:], selv_sb[64:65, 0, :],
                                         stg_even[HD:HD + 1, :],
                                         start=True, stop=False)
                        nc.tensor.matmul(bc[:], selv_sb[64:65, 1, :],
                                         stg_odd[HD:HD + 1, :],
                                         start=False, stop=True)
                        rb = nrmp.tile([64, 2, QC], BF16, tag="rb")
                        with nc.allow_low_precision(reason="1/den in bf16: ~0.2% fro"):
                            nc.vector.reciprocal(rb[:, 0, :], bc[0:64, :])
                            nc.vector.reciprocal(rb[:, 1, :], bc[64:128, :])
                        cx = cxp.tile([128, QC], BF16, tag="ctxn")
                        nc.vector.tensor_mul(cx[0:64, :], stg_even[0:HD, :], rb[:, 0, :])
                        nc.vector.tensor_mul(cx[64:128, :], stg_odd[0:HD, :], rb[:, 1, :])
                        ctxn[pair][j] = cx

                    def po_items(j):
                        posb = pop.tile([128, OCT, QC], BF16, tag="posb")

                        def mk_o(o):
                            def run():
                                ps = outp.tile([128, QC], F32, tag="ops")
                                nc.tensor.matmul(
                                    ps[:], wo_sb[:, 0, o * 128:(o + 1) * 128],
                                    ctxn[0][j][:], start=True, stop=False)
                                nc.tensor.matmul(
                                    ps[:], wo_sb[:, 1, o * 128:(o + 1) * 128],
                                    ctxn[1][j][:], start=False, stop=True)
                                with nc.allow_low_precision(reason="bf16 RS partials"):
                                    nc.vector.tensor_scalar_add(
                                        posb[:, o, :], ps[:], boq_sb[:, o, :])
                            return (0.6, run)

                        def dma_run():
                            # scatter into the straddling RS slabs:
                            # region A (q<1536): 4 x 384; region B: 4 x 128
                            q0, qe = j * QC, (j + 1) * QC
                            q = q0
                            while q < qe:
                                if q < 3 * QC:
                                    slab = q // PWA
                                    end = min((slab + 1) * PWA, 3 * QC, qe)
                                    dst = rs_in[0][slab, :, q - slab * PWA:
                                                   q - slab * PWA + end - q]
                                else:
                                    slab = (q - 3 * QC) // PWB
                                    end = min(3 * QC + (slab + 1) * PWB, qe)
                                    dst = rs_in[1][slab, :, q - 3 * QC - slab * PWB:
                                                   q - 3 * QC - slab * PWB + end - q]
                                nc.sync.dma_start(
                                    dst.rearrange("(o p) q -> p o q", p=128),
                                    posb[:, :, q - q0:end - q0],
                                )
                                q = end

                        for o in range(OCT):
                            fill_hi.append(mk_o(o))
                        fill_hi.append((0.1, dma_run))

                    def rs(h):
                        _ccs.append(nc.gpsimd.collective_compute(
                            "ReduceScatter",
                            mybir.AluOpType.add,
                            replica_groups=[[0, 1, 2, 3], [4, 5, 6, 7]],
                            ins=[rs_in[h][:].opt()],
                            outs=[rs_out[h][:].opt()],
                        ))
                        _rds.append((nc.sync.dma_start(po_out[h][:], rs_out[h][:]), h))

                    # ---- schedule: j-outer, pairs interleaved; projections
                    # and out-proj partials drain through the filler queue so
                    # PE and ACT overlap throughout; RS{j0..j2} fires right
                    # after po(2), RS{j3} is the (smallest possible) tail ----
                    marks_q = {}
                    marks_full = {}
                    for j in range(NQ):
                        for pair in (0, 1):
                            fill_lo.append(proj_qk_item(pair, j, wq_sb, qT_sb))
                            marks_q[(pair, j)] = len(fill_lo)
                            fill_lo.append(proj_qk_item(pair, j, wk_sb, kT_sb))
                            for t in range(4 * j, 4 * j + 4):
                                fill_lo.append(proj_v_item(pair, t))
                            marks_full[(pair, j)] = len(fill_lo)
                    for j in range(NQ):
                        for pair in (0, 1):
                            if j == 0:
                                drain_lo(marks_full[(pair, j)])
                                attn2(pair, j)
                            else:
                                drain_lo(marks_q[(pair, j)])
                                attn2(pair, j, mid_mark=marks_full[(pair, j)])
                        po_items(j)
                        if j == 2:
                            drain_hi()
                            rs(0)
                    drain_hi()
                    drain_lo(len(fill_lo))
                    rs(1)

    upd = _ccs[0].ins.sync_info.on_update[0]
    cc_done_sem = bass.SemaphoreHandle(upd.ant_name, upd.id)
    for rd, h in _rds:
        rd.wait_op(cc_done_sem, h + 1, "sem-ge", check=False)
    nc.compile()
    return nc


def _causal_mask():
    # msk[kp, m, qf] = 1 where (m*128 + kp) <= qf else 0  (keep k <= q)
    kp = np.arange(128)[:, None, None]
    m = np.arange(4)[None, :, None]
    qf = np.arange(QC)[None, None, :]
    return (m * 128 + kp <= qf).astype(ml_dtypes.bfloat16)


def _in_maps(x, Wq, Wk, Wv, Wo, bo):
    bf = ml_dtypes.bfloat16
    msk = _causal_mask()
    selv = np.zeros((1, 2, 128), dtype=bf)
    selv[0, 0, 0:64] = 1.0
    selv[0, 1, 64:128] = 1.0
    boq = (bo.reshape(OCT, 128).T / GROUP).astype(np.float32)[:, :, None]
    xT = [np.ascontiguousarray(x[b].T).astype(bf) for b in range(B)]
    maps = []
    for c in range(NCORES):
        b, g = c // GROUP, c % GROUP
        cs = slice(g * CW, (g + 1) * CW)
        maps.append({
            "xT": xT[b],
            "wq": np.ascontiguousarray(Wq[:, cs]).astype(bf),
            "wk": np.ascontiguousarray(Wk[:, cs]).astype(bf),
            "wv": np.ascontiguousarray(Wv[:, cs]).astype(bf),
            "wo": np.ascontiguousarray(Wo[cs, :]).astype(bf),
            "boq": boq,
            "msk": msk,
            "vones": np.ones((128, NKC, HPC, 1), dtype=bf),
            "selv": selv,
        })
    return maps


def kernel(x, Wq, Wk, Wv, Wo, bo, _trace=False):
    x = np.asarray(x, dtype=np.float32)
    Wq, Wk, Wv, Wo, bo = (np.asarray(a, dtype=np.float32) for a in (Wq, Wk, Wv, Wo, bo))
    if "nc" not in _CACHE:
        _CACHE["nc"] = _build_bass()
    nc = _CACHE["nc"]
    res = run_bass_kernel_spmd(
        nc, _in_maps(x, Wq, Wk, Wv, Wo, bo), list(range(NCORES)), trace=_trace
    )
    out = np.zeros((B, S, D), dtype=np.float32)
    for c in range(NCORES):
        b, g = c // GROUP, c % GROUP
        pa = np.asarray(res.results[c]["po0"]).astype(np.float32)
        out[b, g * PWA:(g + 1) * PWA, :] = pa.T
        pb = np.asarray(res.results[c]["po1"]).astype(np.float32)
        q0 = 3 * QC + g * PWB
        out[b, q0:q0 + PWB, :] = pb.T
    if _trace:
        return out, res
    return out
